# revision 9
# baseline (speedup 1.0000x reference)
"""Trainium2 Bass kernel for 3x3 same-padding Conv2d (B=16, C=256, H=W=112) + bias.

Strategy: data-parallel over batch (2 images per core on 8 NeuronCores), with
1D Winograd F(4,3) along y (2.25x fewer MACs in y) and direct 3-tap conv along
x via shifted slices. All heavy compute stays on the tensor engine:

  stage 1 (y-transform, PE): per x-column, a data-stationary matmul
      lhsT = input column [114 y-rows, 128 ch] (bf16),
      rhs  = constant windowed-B^T matrix [114, 6i*28t] (bf16)
    -> V[c, i, t] for that x. Zero-padding at y=-1/112 is folded into the
    constant matrix (its pad rows are zero), so staging rows can hold garbage.
  stage 2 (GEMM, PE): M_i[co, t, x] = sum_{c,kx} U[i,kx][c,co] * V_i[c,t,x+kx-1]
    as fp16 matmuls accumulating over (kx, ch-half) in PSUM; U = G @ W
    pre-transformed on the host. 6 points * 6 accum vs direct conv's 18 taps
    per 4 output rows -> 2x fewer PE streaming cycles overall.
  stage 3 (inverse A^T, DVE+ACT): 13 fp16 vector ops (CSE'd) combine the 6
    M_i planes into 4 output rows; scalar engine adds bias and converts to
    fp32 while draining; sync-ring DMA scatters rows y=4t+r to DRAM.

Images are processed in x-halves (56 output columns + halo) to fit SBUF.
PSUM drains: stage-1 V on ACT, GEMM M on ACT, inverse on DVE.
"""
import numpy as np
import ml_dtypes

from concourse import bacc, bass, mybir, tile
from concourse.bass_utils import run_bass_kernel_spmd

B, C, H, Wd = 16, 256, 112, 112
NCORES = 8
BPC = B // NCORES        # images per core
T = 28                   # y tiles (4 rows each)
NI = 6                   # winograd points per tile
WCOL = NI * T            # 168 columns of the windowed-B^T constant
NROW = H + 2             # 114 staging rows (y = -1 .. 112)
XH = Wd // 2             # 56 output columns per x-half
VSLOT = 59               # V x-slots per half (58 data/halo + 1 zero border)
NTX = T * XH             # 1568 (t, x) positions per half
f32 = mybir.dt.float32
bf16 = mybir.dt.bfloat16
fp16 = mybir.dt.float16

TCHUNKS = [(0, 8), (8, 8), (16, 8), (24, 4)]   # t-chunks for GEMM psum groups

BT_M = np.array([
    [4, 0, -5, 0, 1, 0],
    [0, -4, -4, 1, 1, 0],
    [0, 4, -4, -1, 1, 0],
    [0, -2, -1, 2, 1, 0],
    [0, 2, -1, -2, 1, 0],
    [0, 4, 0, -5, 0, 1]], dtype=np.float64)
G_M = np.array([
    [1 / 4, 0, 0],
    [-1 / 6, -1 / 6, -1 / 6],
    [-1 / 6, 1 / 6, -1 / 6],
    [1 / 24, 1 / 12, 1 / 6],
    [1 / 24, -1 / 12, 1 / 6],
    [0, 0, 1]], dtype=np.float64)
AT_M = np.array([
    [1, 1, 1, 1, 1, 0],
    [0, 1, -1, 2, -2, 0],
    [0, 1, 1, 4, 4, 0],
    [0, 1, -1, 8, -8, 1]], dtype=np.float64)


def build(repeat: int = 1, no_in: bool = False, no_s1: bool = False,
          no_gemm: bool = False, no_inv: bool = False,
          rhs_contig: bool = False, no_out: bool = False):
    nc = bacc.Bacc("TRN2", debug=False)
    inp_d = nc.dram_tensor("inp", [BPC, C, H, Wd], bf16, kind="ExternalInput").ap()
    u_d = nc.dram_tensor("u", [NI, 3, C, C], fp16, kind="ExternalInput").ap()
    wbt_d = nc.dram_tensor("wbt", [NROW, WCOL], bf16, kind="ExternalInput").ap()
    bias_d = nc.dram_tensor("bias", [C, 1], f32, kind="ExternalInput").ap()
    out_d = nc.dram_tensor("out", [BPC, C, H, Wd], f32, kind="ExternalOutput").ap()

    with tile.TileContext(nc) as tc:
        with (
            tc.tile_pool(name="wp", bufs=1) as wp,
            tc.tile_pool(name="mp", bufs=2) as mp,
            tc.tile_pool(name="op", bufs=1) as op,
            tc.tile_pool(name="pp", bufs=1, space=bass.MemorySpace.PSUM) as pp,
        ):
            # constants
            wbt_t = wp.tile([NROW, WCOL], bf16, name="wbt")
            nc.sync.dma_start(wbt_t[:], wbt_d[:, :])
            ut = {}
            for i in range(NI):
                for kx in range(3):
                    for kh in range(2):
                        for mh in range(2):
                            t = wp.tile([128, 128], fp16, name=f"u{i}{kx}{kh}{mh}")
                            nc.sync.dma_start(
                                t[:], u_d[i, kx, kh * 128:(kh + 1) * 128,
                                          mh * 128:(mh + 1) * 128])
                            ut[i, kx, kh, mh] = t
            biases = []
            for mh in range(2):
                bt = wp.tile([128, 1], f32, name=f"bias{mh}")
                nc.sync.dma_start(bt[:], bias_d[mh * 128:(mh + 1) * 128, :])
                biases.append(bt)

            # V buffers [c, (slot, i, t)], fp16; slots 0 and 58 are persistent
            # zero borders (only drains to slots 1..57 ever happen)
            zt = wp.tile([128, 1], f32, name="zsrc")
            nc.vector.memset(zt[:], 0.0)
            vbufs = []
            for kh in range(2):
                v = wp.tile([128, VSLOT * WCOL], fp16, name=f"v{kh}")
                nc.vector.tensor_copy(v[:, 0:WCOL], zt[:].to_broadcast((128, WCOL)))
                nc.vector.tensor_copy(
                    v[:, (VSLOT - 1) * WCOL:VSLOT * WCOL],
                    zt[:].to_broadcast((128, WCOL)))
                vbufs.append(v)

            # input staging: [y-slot 114, (c 256, x 112)] bf16, reused per image
            stg = wp.tile([NROW, C * Wd], bf16, name="stg")
            # pad rows (partitions 112/113) must be finite: 0*NaN=NaN in the
            # matmul even though the windowed-B^T pad rows are zero. Zero the
            # 32-aligned partition tail once; the DMA rewrites 96..111 each
            # image but never touches 112/113, so the zeros persist.
            nc.vector.tensor_copy(
                stg[96:NROW, :],
                zt[96:NROW, :].to_broadcast((NROW - 96, C * Wd)))

            def body():
                for img in range(BPC):
                    # row y -> partition y; pad partitions 112/113 stay zero
                    for kh in range(2):
                        if no_in:
                            continue
                        nc.gpsimd.dma_start(
                            stg[0:H, :].rearrange("p (c x) -> p c x", x=Wd)
                            [:, kh * 128:(kh + 1) * 128, :],
                            inp_d[img, kh * 128:(kh + 1) * 128]
                            .rearrange("c y x -> y c x"))
                    stg_v = stg[:].rearrange("p (c x) -> p c x", x=Wd)
                    for xh in range(2):
                        xo = xh * XH
                        # stage 1: x-columns feeding this half's V slots.
                        # half 0: x 0..56 -> slots 1..57 ; half 1: x 55..111
                        # -> slots 1..57 (slot = x - xo + 1 + xh)
                        xs = [x for x in range(xo - 1, xo + XH + 1)
                              if 0 <= x < Wd]
                        for kh in range(2):
                            if no_s1:
                                continue
                            bi = 0
                            while bi < len(xs):
                                nb = min(3, len(xs) - bi)
                                ps = pp.tile([128, 3 * WCOL], f32,
                                             name="s1", tag="s1", bufs=2)
                                for j in range(nb):
                                    x = xs[bi + j]
                                    nc.tensor.matmul(
                                        ps[:, j * WCOL:(j + 1) * WCOL],
                                        stg_v[:, kh * 128:(kh + 1) * 128, x],
                                        wbt_t[:],
                                        start=True, stop=True)
                                s0 = xs[bi] - xo + 1 + xh
                                nc.scalar.copy(
                                    vbufs[kh][:, s0 * WCOL:(s0 + nb) * WCOL],
                                    ps[:, :nb * WCOL])
                                bi += nb
                        # stage 2 GEMM + stage 3 inverse, per out-channel half
                        for mh in range(2):
                            mbuf = mp.tile([128, NI * NTX], fp16,
                                           name="m", tag="m")
                            for i in range(NI):
                                if no_gemm:
                                    continue
                                for (t0, nt) in TCHUNKS:
                                    ps = pp.tile([128, 8 * XH], f32,
                                                 name="g", tag="g", bufs=3)
                                    for a, (kx, kh) in enumerate(
                                            [(kx, kh) for kx in range(3)
                                             for kh in range(2)]):
                                        if rhs_contig:
                                            rhs = vbufs[kh][:, :nt * XH]
                                        else:
                                            rhs = (vbufs[kh][:]
                                                   .rearrange(
                                                       "p (s w) -> p s w",
                                                       w=WCOL)
                                                   [:, kx + xh:kx + xh + XH,
                                                    i * T + t0:i * T + t0 + nt]
                                                   .rearrange("p x t -> p t x"))
                                        nc.tensor.matmul(
                                            ps[:, :nt * XH],
                                            ut[i, kx, kh, mh][:],
                                            rhs,
                                            start=(a == 0), stop=(a == 5))
                                    nc.scalar.copy(
                                        mbuf[:, i * NTX + t0 * XH:
                                             i * NTX + (t0 + nt) * XH],
                                        ps[:, :nt * XH])
                            # stage 3: inverse A^T on DVE (fp16), bias on ACT
                            if no_inv:
                                continue
                            mv = mbuf[:].rearrange("p (i n) -> p i n", n=NTX)
                            ta = op.tile([128, NTX], fp16, name="ta", tag="ta")
                            tb = op.tile([128, NTX], fp16, name="tb", tag="tb")
                            tc_ = op.tile([128, NTX], fp16, name="tc", tag="tc")
                            td = op.tile([128, NTX], fp16, name="td", tag="td")
                            te = op.tile([128, NTX], fp16, name="te", tag="te")
                            mult = mybir.AluOpType.mult
                            add = mybir.AluOpType.add
                            vv = nc.vector
                            vv.tensor_add(ta[:], mv[:, 1], mv[:, 2])
                            vv.tensor_sub(tb[:], mv[:, 1], mv[:, 2])
                            vv.tensor_add(tc_[:], mv[:, 3], mv[:, 4])
                            vv.tensor_sub(td[:], mv[:, 3], mv[:, 4])
                            rts = []
                            for r in range(4):
                                rt = op.tile([128, NTX], fp16,
                                             name=f"r{r}", tag=f"r{r}")
                                rts.append(rt)
                            # r0 = M0 + ta + tc
                            vv.tensor_add(te[:], mv[:, 0], ta[:])
                            vv.tensor_add(rts[0][:], te[:], tc_[:])
                            # r1 = 2*td + tb ; r2 = 4*tc + ta
                            vv.scalar_tensor_tensor(
                                rts[1][:], td[:], 2.0, tb[:], mult, add)
                            vv.scalar_tensor_tensor(
                                rts[2][:], tc_[:], 4.0, ta[:], mult, add)
                            # r3 = 8*td + tb + M5
                            vv.scalar_tensor_tensor(
                                te[:], td[:], 8.0, tb[:], mult, add)
                            vv.tensor_add(rts[3][:], te[:], mv[:, 5])
                            for r in range(4):
                                ot = op.tile([128, NTX], f32,
                                             name="ot", tag="ot", bufs=2)
                                nc.scalar.add(ot[:], rts[r][:], biases[mh][:])
                                if no_out:
                                    continue
                                nc.sync.dma_start(
                                    out_d[img, mh * 128:(mh + 1) * 128]
                                    .rearrange("p (t q) x -> p t q x", q=4)
                                    [:, :, r, xo:xo + XH],
                                    ot[:].rearrange("p (t x) -> p t x", x=XH))

            if repeat > 1:
                with tc.For_i(0, repeat, 1):
                    body()
            else:
                body()

    nc.compile()
    return nc


def _host_inputs(inp, W, bias):
    """Host-side pretransforms shared by kernel() and test.py."""
    inp_bf = np.asarray(inp, dtype=np.float32).astype(ml_dtypes.bfloat16)
    u = np.einsum("iy,ocyx->ixco", G_M,
                  np.asarray(W, dtype=np.float64)).astype(np.float16)
    wbt = np.zeros((NROW, WCOL), dtype=np.float64)
    for t in range(T):
        for i in range(NI):
            for yl in range(NI):
                y = 4 * t - 1 + yl
                if 0 <= y < H:
                    wbt[y, i * T + t] += BT_M[i, yl]
    wbt = wbt.astype(ml_dtypes.bfloat16)
    bias_r = np.ascontiguousarray(
        np.asarray(bias, dtype=np.float32).reshape(C, 1))
    return inp_bf, u, wbt, bias_r


_NC = None


def kernel(inp, W, bias):
    global _NC
    if _NC is None:
        _NC = build()
    inp_bf, u, wbt, bias_r = _host_inputs(inp, W, bias)
    in_maps = [
        {"inp": np.ascontiguousarray(inp_bf[c * BPC:(c + 1) * BPC]),
         "u": u, "wbt": wbt, "bias": bias_r}
        for c in range(NCORES)
    ]
    res = run_bass_kernel_spmd(_NC, in_maps, list(range(NCORES)))
    return np.concatenate(
        [res.results[c]["out"] for c in range(NCORES)], axis=0)
